# revision 1
# baseline (speedup 1.0000x reference)
"""Trainium2 Bass kernel for nn_DebugBertSelfAttention_87093346828836.

The reference module is a debug variant of BERT self-attention: after the
Q/K/V projections it overwrites q, k, v with the constant 0.01.  With
uniform q/k, every attention score is identical, so softmax yields uniform
probabilities (1/SEQ), and the context is the mean of the constant v —
i.e. every output element equals the same f32 constant, independent of all
inputs.  The f32-accumulated value (matching the XLA CPU reference) is
0x3c23d739 = 0.010000044.

The kernel therefore reduces to materializing the (8, 1024, 1024) constant
output.  Sharding: batch across the 8 cores — each core writes one
1024x1024 f32 block (4 MiB).  On device: DVE memsets an SBUF tile with the
constant, then HWDGE DMAs replicate it into the core's output DRAM buffer.
The host concatenates the 8 per-core blocks into the full output.
"""

import numpy as np

NUM_CORES = 8
BATCH, SEQ, HIDDEN = 8, 1024, 1024
OUT_SHAPE = (BATCH, SEQ, HIDDEN)

# Per-core output block: 1024*1024 f32 = 4 MiB, viewed as [128, 8192].
P = 128
F = (SEQ * HIDDEN) // P  # 8192

# SBUF staging tile: [128, CHUNK] f32, replicated F//CHUNK times by DMA.
CHUNK = 1024

# f32 bits of the reference output constant (see module docstring).
CONST_BITS = 0x3C23D739
CONST = float(np.uint32(CONST_BITS).view(np.float32))


def build_nc():
    """Build the per-core Bass program (identical on all cores)."""
    from concourse import bass
    from concourse import mybir

    nc = bass.Bass(target_bir_lowering=False)
    out = nc.dram_tensor("out", [P, F], mybir.dt.float32, kind="ExternalOutput")

    with (
        nc.Block() as block,
        nc.semaphore("msem") as msem,
        nc.semaphore("dsem") as dsem,
        nc.sbuf_tensor("buf", [P, CHUNK], mybir.dt.float32) as buf,
    ):

        @block.vector
        def _(vector):
            vector.memset(buf[:, :], CONST).then_inc(msem, 1)

        @block.sync
        def _(sync):
            sync.wait_ge(msem, 1)
            n = F // CHUNK
            for j in range(n):
                sync.dma_start(
                    out[:, j * CHUNK : (j + 1) * CHUNK], buf[:, :]
                ).then_inc(dsem, 16)
            sync.wait_ge(dsem, 16 * n)

    return nc


def kernel(**inputs) -> np.ndarray:
    from concourse.bass_utils import run_bass_kernel_spmd

    nc = build_nc()
    in_maps = [{} for _ in range(NUM_CORES)]
    res = run_bass_kernel_spmd(nc, in_maps, list(range(NUM_CORES)))

    out = np.empty(OUT_SHAPE, np.float32)
    for i in range(NUM_CORES):
        out[i] = res.results[i]["out"].reshape(SEQ, HIDDEN)
    return out


# revision 2
# speedup vs baseline: 1.0083x; 1.0083x over previous
"""Trainium2 Bass kernel for nn_DebugBertSelfAttention_87093346828836.

The reference module is a debug variant of BERT self-attention: after the
Q/K/V projections it overwrites q, k, v with the constant 0.01.  With
uniform q/k, every attention score is identical, so softmax yields uniform
probabilities (1/SEQ), and the context is the mean of the constant v —
i.e. every output element equals the same f32 constant, independent of all
inputs.  The f32-accumulated value (matching the XLA CPU reference) is
0x3c23d739 = 0.010000044.

The kernel therefore reduces to materializing the (8, 1024, 1024) constant
output.  Sharding: batch across the 8 cores — each core writes one
1024x1024 f32 block (4 MiB).  On device: DVE memsets an SBUF tile with the
constant, then HWDGE DMAs replicate it into the core's output DRAM buffer.
The host concatenates the 8 per-core blocks into the full output.
"""

import numpy as np

NUM_CORES = 8
BATCH, SEQ, HIDDEN = 8, 1024, 1024
OUT_SHAPE = (BATCH, SEQ, HIDDEN)

# Per-core output block: 1024*1024 f32 = 4 MiB, viewed as [128, 8192].
P = 128
F = (SEQ * HIDDEN) // P  # 8192

# SBUF staging tile: [128, CHUNK] f32, replicated F//CHUNK times by DMA.
CHUNK = 1024

# f32 bits of the reference output constant (see module docstring).
CONST_BITS = 0x3C23D739
CONST = float(np.uint32(CONST_BITS).view(np.float32))


def build_nc():
    """Build the per-core Bass program (identical on all cores)."""
    from concourse import bass
    from concourse import mybir

    nc = bass.Bass(target_bir_lowering=False)
    out = nc.dram_tensor("out", [P, F], mybir.dt.float32, kind="ExternalOutput")

    n = F // CHUNK  # total DMA count

    with (
        nc.Block(no_gpsimd_drain=True) as block,
        nc.semaphore("msem") as msem,
        nc.semaphore("dsem") as dsem,
        nc.sbuf_tensor("buf", [P, CHUNK], mybir.dt.float32) as buf,
    ):

        @block.gpsimd
        def _(gpsimd):
            gpsimd.memset(buf[:, :], CONST).then_inc(msem, 1)

        # Split DMA issue across both HWDGE engines (SP + ACT): two
        # descriptor rings in parallel halve issue serialization.
        @block.sync
        def _(sync):
            sync.wait_ge(msem, 1)
            for j in range(0, n, 2):
                sync.dma_start(
                    out[:, j * CHUNK : (j + 1) * CHUNK], buf[:, :]
                ).then_inc(dsem, 16)
            sync.wait_ge(dsem, 16 * n)

        @block.scalar
        def _(scalar):
            scalar.wait_ge(msem, 1)
            for j in range(1, n, 2):
                scalar.dma_start(
                    out[:, j * CHUNK : (j + 1) * CHUNK], buf[:, :]
                ).then_inc(dsem, 16)
            scalar.wait_ge(dsem, 16 * n)

    return nc


def kernel(**inputs) -> np.ndarray:
    from concourse.bass_utils import run_bass_kernel_spmd

    nc = build_nc()
    in_maps = [{} for _ in range(NUM_CORES)]
    res = run_bass_kernel_spmd(nc, in_maps, list(range(NUM_CORES)))

    out = np.empty(OUT_SHAPE, np.float32)
    for i in range(NUM_CORES):
        out[i] = res.results[i]["out"].reshape(SEQ, HIDDEN)
    return out
